# revision 28
# baseline (speedup 1.0000x reference)
"""Distributed Trainium2 kernel for a 16-head attention layer.

Problem: B=2, L=2048, HID=1024, H=16 (torch-Linear projections, masked softmax).
Sharding: 8 cores = batch (2) x query-chunk (4). Each core computes the Q
projection for its 512-query chunk, attention for all 16 heads over its
queries, the output projection for its chunk, and the full K/V projections
for its batch (duplicated within the 4-core group -- collectives on this
harness pay the full cross-core launch skew, measured ~30us per call, so
the kernel stays embarrassingly parallel).

Optimizations over the v0 baseline:
- Masked-key compaction: mask is [B,1,1,L] and masked keys score -10000 ->
  exp underflows to exactly 0 in f32 (same as the reference's softmax), so
  they contribute nothing to numerator or denominator. The host compacts
  K/V/mask to the kept keys (padded to a multiple of 128; pad slots get the
  -10000 bias). With the ~Bernoulli(0.5) mask this halves K/V projection,
  QK, PV and ScalarE-exp work. The Bass program is compiled per
  key-tile-count (LTK) and cached; LTK > 10 falls back to an unsharded
  upfront schedule that fits SBUF at any key count.
- Pipelined attention: QK/exp for pair hp+1 is emitted before the PV
  accumulation of pair hp (lag-1 ladder), so the in-order engine queues
  pipeline the QK -> exp -> PV chain instead of serializing it; the
  normalize tail of each pair is deferred into the next pair's PV phase so
  the PE never stalls on the DVE reciprocal chain.
- Normalize tail: denominators (PSUM row 64 of the PV accumulators, from
  the ones-column trick) are staged to SBUF, inverted with the custom-DVE
  reciprocal_approx_fast (~5x faster than the iterative divide), and
  broadcast across the 64 head dims with two concurrent column-tiled fp16
  rank-1 matmuls (the v0 fp32 ones-matmuls ran at quarter rate).

Layout strategy: all matmul contractions need the contracted dim on SBUF
partitions, so activations and weights are fed pre-transposed (host-side
numpy transpose + bf16 cast during sharding):
  qT [HID, LQ]  kT/vT [HID, LK]   W*T [HID, HID] (= W.T)
Scores are computed transposed (S.T[lk, lq]) so the mask bias is a
per-partition bias fused into the ScalarE exp, and PV consumes P.T directly.
A ones-column appended to each V tile makes the PV matmul emit the softmax
denominators as PSUM row 64.
"""

import sys
import types

import numpy as np
import ml_dtypes

# ---- problem constants (hardcoded; kernel.py must be self-contained) ----
B, L, HID, H = 2, 2048, 1024, 16
DH = HID // H          # 64
N_CORES = 8
GSZ = N_CORES // B     # 4 cores per batch group
LQ = (B * L) // N_CORES  # 512 queries per core
P = 128
KT = HID // P          # 8 contraction tiles
OT = HID // P          # 8 output tiles
NPAIR = H // 2         # 8 head pairs
SCALE = DH ** -0.5
BF16 = ml_dtypes.bfloat16
VW = DH + 1            # 65: per-head V columns + ones column for denominator


def _ensure_profile_hook():
    """Install the NTFF profiling hook trn_boot couldn't (antenv.axon_hooks
    is missing from the image); harmless if profiling is never requested."""
    if "antenv.axon_hooks" in sys.modules:
        return
    try:
        from trn_agent_boot.trn_boot import _ntff_profile_via_ctypes

        hook = _ntff_profile_via_ctypes("/opt/axon/libaxon_pjrt.so")
    except Exception:
        hook = None
    mod = types.ModuleType("antenv.axon_hooks")
    mod.get_axon_ntff_profile_hook = lambda: hook
    mod.set_axon_ntff_profile_hook = lambda h: None
    sys.modules["antenv.axon_hooks"] = mod


def build_bass(LTK):
    """Build + compile the per-core Bass program for LTK key tiles
    (LK = 128*LTK compacted keys; same graph on all 8 cores)."""
    import concourse.mybir as mybir
    import concourse.tile as tile
    from concourse import bacc
    import contextlib

    LK = LTK * P
    interleaved = LTK <= 10
    f32 = mybir.dt.float32
    bf16 = mybir.dt.bfloat16
    fp16 = mybir.dt.float16
    ADD = mybir.AluOpType.add
    MULT = mybir.AluOpType.mult
    BYPASS = mybir.AluOpType.bypass
    EXP = mybir.ActivationFunctionType.Exp
    RG = [list(range(g * GSZ, (g + 1) * GSZ)) for g in range(B)]

    nc = bacc.Bacc("TRN2", target_bir_lowering=False, debug=False, num_devices=N_CORES)

    qT = nc.declare_dram_parameter("qT", [HID, LQ], bf16, isOutput=False)
    kT = nc.declare_dram_parameter("kT", [HID, LK], bf16, isOutput=False)
    vT = nc.declare_dram_parameter("vT", [HID, LK], bf16, isOutput=False)
    WqT = nc.declare_dram_parameter("WqT", [HID, HID], bf16, isOutput=False)
    WvT = nc.declare_dram_parameter("WvT", [HID, HID], bf16, isOutput=False)
    WoT = nc.declare_dram_parameter("WoT", [HID, HID], bf16, isOutput=False)
    bq = nc.declare_dram_parameter("bq", [P, OT], f32, isOutput=False)
    bo = nc.declare_dram_parameter("bo", [P, OT], f32, isOutput=False)
    bv_row = nc.declare_dram_parameter("bv_row", [1, HID], bf16, isOutput=False)
    maskb = nc.declare_dram_parameter("maskb", [P, LTK], f32, isOutput=False)
    out = nc.declare_dram_parameter("out", [HID, LQ], f32, isOutput=True)
    WkT = nc.declare_dram_parameter("WkT", [HID, HID], bf16, isOutput=False)
    bk = nc.declare_dram_parameter("bk", [P, OT], f32, isOutput=False)

    with tile.TileContext(nc) as tc:
        with contextlib.ExitStack() as _stk:
            pool = lambda *a, **k: _stk.enter_context(tc.tile_pool(*a, **k))
            consts = pool(name="consts", bufs=1)
            khT_p = pool(name="khT", bufs=OT)
            vhx_p = pool(name="vhx", bufs=LTK)
            qhT_p = pool(name="qhT", bufs=OT)
            attnT_p = pool(name="attnT", bufs=NPAIR)
            osb_p = pool(name="osb", bufs=2)
            psum = pool(name="psum", bufs=1, space="PSUM")

            # ---- constants ----
            maskb_sb = consts.tile([P, LTK], f32)
            nc.sync.dma_start(maskb_sb[:], maskb[:])
            bq_sb = consts.tile([P, OT], f32, tag="bq")
            nc.sync.dma_start(bq_sb[:], bq[:])
            bo_sb = consts.tile([P, OT], f32, tag="bo")
            nc.sync.dma_start(bo_sb[:], bo[:])
            bvr_sb = consts.tile([1, HID], bf16, tag="bvr")
            nc.sync.dma_start(bvr_sb[:], bv_row[:])
            ones_bf = consts.tile([1, P], bf16, tag="ones_bf")
            nc.vector.memset(ones_bf[:], 1.0)
            ones16 = consts.tile([1, DH], fp16, tag="ones16")
            nc.vector.memset(ones16[:], 1.0)
            bk_sb = consts.tile([P, OT], f32, tag="bk")
            nc.sync.dma_start(bk_sb[:], bk[:])

            # bias-v broadcast tile [128, HID] (bias along free dim needs a
            # full tile; built once via a rank-1 ones @ bv_row matmul)
            bvb_ps = psum.tile([P, 1024], f32, tag="mm", bufs=2)
            for h2 in range(2):
                nc.tensor.matmul(
                    bvb_ps[:, h2 * 512 : (h2 + 1) * 512],
                    ones_bf[:, :],
                    bvr_sb[:, h2 * 512 : (h2 + 1) * 512],
                    start=True,
                    stop=True,
                )
            bvb = consts.tile([P, HID], f32, tag="bvb")
            nc.vector.tensor_copy(bvb[:], bvb_ps[:])

            khT = [None] * OT
            vhx = [None] * LTK
            qhT = [None] * OT
            attnT = [None] * NPAIR

            def load(pool_, tag, dramp, width):
                tiles = []
                for i in range(KT):
                    w = pool_.tile([P, width], bf16, tag=tag, name=f"{tag}{i}")
                    nc.sync.dma_start(w[:], dramp[i * P : (i + 1) * P, :])
                    tiles.append(w)
                return tiles

            def emit_qproj(wq_sb, qT_sb):
                for ot in range(OT):
                    ps = psum.tile([P, 1024], f32, tag="mm", bufs=2,
                                   name=f"qps{ot}")
                    for i in range(KT):
                        nc.tensor.matmul(
                            ps[:, 0:LQ],
                            wq_sb[i][:, ot * P : (ot + 1) * P],
                            qT_sb[i][:, :],
                            start=(i == 0),
                            stop=(i == KT - 1),
                        )
                    t = qhT_p.tile([P, LQ], bf16, tag="qhT")
                    nc.vector.tensor_scalar(
                        t[:], ps[:, 0:LQ], bq_sb[:, ot : ot + 1], None, op0=ADD
                    )
                    qhT[ot] = t

            def emit_kproj(ot, wcol, bcol, wk_sb, kT_sb, dst_p, dst_tag):
                """One khT-style output tile [128, LK] bf16 from weight
                column block `wcol` of wk_sb, bias column `bcol`."""
                t = dst_p.tile([P, LK], bf16, tag=dst_tag, name=f"{dst_tag}{ot}")
                for c0 in range(0, LK, 1024):
                    w = min(1024, LK - c0)
                    ps = psum.tile([P, 1024], f32, tag="mm", bufs=2,
                                   name=f"kps{dst_tag}{ot}_{c0}")
                    for h0 in range(0, w, 512):
                        hw = min(512, w - h0)
                        for i in range(KT):
                            nc.tensor.matmul(
                                ps[:, h0 : h0 + hw],
                                wk_sb[i][:, wcol * P : (wcol + 1) * P],
                                kT_sb[i][:, c0 + h0 : c0 + h0 + hw],
                                start=(i == 0),
                                stop=(i == KT - 1),
                            )
                    nc.vector.tensor_scalar(
                        t[:, c0 : c0 + w],
                        ps[:, 0:w],
                        bk_sb[:, bcol : bcol + 1],
                        None,
                        op0=ADD,
                    )
                return t

            def emit_vstep(j, wv_sb, vT_sb):
                """vhx[j] [128, 16*65] bf16 (keys on partitions, per-head
                64 value cols + ones col)."""
                ps = psum.tile([P, 1024], f32, tag="mm", bufs=2, name=f"vps{j}")
                for half in range(2):
                    for i in range(KT):
                        nc.tensor.matmul(
                            ps[:, half * 512 : (half + 1) * 512],
                            vT_sb[i][:, j * P : (j + 1) * P],
                            wv_sb[i][:, half * 512 : (half + 1) * 512],
                            start=(i == 0),
                            stop=(i == KT - 1),
                        )
                t = vhx_p.tile([P, H * VW], bf16, tag="vhx", name=f"vhx{j}")
                ps3 = ps[:].rearrange("p (h d) -> p h d", d=DH)
                out3 = t[:].rearrange("p (h w) -> p h w", w=VW)[:, :, 0:DH]
                bvb3 = bvb[:].rearrange("p (h d) -> p h d", d=DH)
                nc.vector.tensor_tensor(out3, ps3, bvb3, op=ADD)
                ones3 = t[:].rearrange("p (h w) -> p h w", w=VW)[:, :, DH:VW]
                nc.vector.memset(ones3, 1.0)
                vhx[j] = t

            # ---- attention: QK/exp runs a one-pair ladder ahead of PV ----
            pts = {}

            def emit_qkexp(hp, vstep=None):
                for j in range(LTK):
                    if vstep is not None:
                        vstep(j)
                    sp = psum.tile([P, 1024], f32, tag="mm", bufs=2,
                                   name=f"sp{hp}_{j}")
                    nc.tensor.matmul(
                        sp[:, 0:512],
                        khT[hp][0:DH, j * P : (j + 1) * P],
                        qhT[hp][0:DH, :],
                        start=True,
                        stop=True,
                    )
                    nc.tensor.matmul(
                        sp[:, 512:1024],
                        khT[hp][DH:P, j * P : (j + 1) * P],
                        qhT[hp][DH:P, :],
                        start=True,
                        stop=True,
                    )
                    pt = pt_p.tile([P, 1024], bf16, tag="pt", name=f"pt{hp}_{j}")
                    nc.scalar.activation(
                        pt[:], sp[:], EXP,
                        bias=maskb_sb[:, j : j + 1], scale=SCALE,
                    )
                    pts[(hp, j)] = pt

            def make_tail(hp, pv0, pv1):
                """Deferred normalize tail for pair hp (emitted mid-way
                through the next pair's PV phase so the in-order PE queue
                never stalls on the DVE reciprocal chain)."""
                def tail():
                    dsb = nrm_p.tile([1, 2 * LQ], f32, tag="dsb",
                                     name=f"dsbh{hp}")
                    nc.vector.tensor_copy(dsb[:, 0:LQ], pv0[DH : DH + 1, :])
                    nc.vector.tensor_copy(dsb[:, LQ : 2 * LQ],
                                          pv1[DH : DH + 1, :])
                    rc32 = nrm_p.tile([1, 2 * LQ], f32, tag="rc32",
                                      name=f"rc32h{hp}")
                    nc.vector.reciprocal_approx_fast(rc32[:], dsb[:])
                    rc16 = nrm_p.tile([1, 2 * LQ], fp16, tag="rc16",
                                      name=f"rc16h{hp}")
                    nc.vector.tensor_copy(rc16[:], rc32[:])
                    rb = psum.tile([P, 1024], f32, tag="mm", bufs=2,
                                   name=f"rb{hp}")
                    nc.tensor.matmul(
                        rb[0:DH, 0:LQ], ones16[:, :], rc16[:, 0:LQ],
                        start=True, stop=True, tile_position=(0, 0),
                    )
                    nc.tensor.matmul(
                        rb[DH:P, 0:LQ], ones16[:, :], rc16[:, LQ : 2 * LQ],
                        start=True, stop=True, tile_position=(0, DH),
                    )
                    # DVE has a single PSUM port: stage the broadcast in SBUF
                    # so the normalize mult reads one PSUM operand
                    rbs = nrm_p.tile([P, LQ], f32, tag="rbs", name=f"rbs{hp}")
                    nc.vector.tensor_copy(rbs[:], rb[:, 0:LQ])
                    at = attnT_p.tile([P, LQ], bf16, tag="attnT",
                                      name=f"at{hp}")
                    nc.vector.tensor_tensor(
                        at[0:DH, :], pv0[0:DH, :], rbs[0:DH, :], op=MULT
                    )
                    nc.vector.tensor_tensor(
                        at[DH:P, :], pv1[0:DH, :], rbs[DH:P, :], op=MULT
                    )
                    attnT[hp] = at
                return tail

            def emit_pv(hp, prev_tail=None):
                pv0 = psum.tile([VW, LQ], f32, tag="pv", bufs=4,
                                name=f"pv0h{hp}")
                pv1 = psum.tile([VW, LQ], f32, tag="pv", bufs=4,
                                name=f"pv1h{hp}")
                tail_j = min(2, LTK - 1)
                for j in range(LTK):
                    pt = pts.pop((hp, j))
                    nc.tensor.matmul(
                        pv0[:, :],
                        vhx[j][:, (2 * hp) * VW : (2 * hp + 1) * VW],
                        pt[:, 0:512],
                        start=(j == 0),
                        stop=(j == LTK - 1),
                    )
                    nc.tensor.matmul(
                        pv1[:, :],
                        vhx[j][:, (2 * hp + 1) * VW : (2 * hp + 2) * VW],
                        pt[:, 512:1024],
                        start=(j == 0),
                        stop=(j == LTK - 1),
                    )
                    if j == tail_j and prev_tail is not None:
                        prev_tail()
                return make_tail(hp, pv0, pv1)

            def emit_oproj(wo_sb):
                for ot in range(OT):
                    ps = psum.tile([P, 1024], f32, tag="mm", bufs=2,
                                   name=f"ops{ot}")
                    for i in range(KT):
                        nc.tensor.matmul(
                            ps[:, 0:LQ],
                            wo_sb[i][:, ot * P : (ot + 1) * P],
                            attnT[i][:, :],
                            start=(i == 0),
                            stop=(i == KT - 1),
                        )
                    o = osb_p.tile([P, LQ], f32, tag="osb")
                    nc.vector.tensor_scalar(
                        o[:], ps[:, 0:LQ], bo_sb[:, ot : ot + 1], None, op0=ADD
                    )
                    nc.sync.dma_start(out[ot * P : (ot + 1) * P, :], o[:])

            if interleaved:
                # Q projection first (its 3MB of inputs gate the first PE
                # work); scoped pools so the stack allocator reuses their
                # SBUF region for the pt/nrm pools opened after
                wk_p = pool(name="wk", bufs=KT)
                kT_p = pool(name="kin", bufs=KT)
                wv_p = pool(name="wv", bufs=KT)
                vT_p = pool(name="vin", bufs=KT)
                with (
                    tc.tile_pool(name="wq", bufs=KT) as wq_p,
                    tc.tile_pool(name="qTin", bufs=KT) as qT_p,
                ):
                    wq_sb = load(wq_p, "wq", WqT, HID)
                    qT_sb = load(qT_p, "qTin", qT, LQ)
                    wk_sb = load(wk_p, "wk", WkT, HID)
                    kT_sb = load(kT_p, "kTin", kT, LK)
                    wv_sb = load(wv_p, "wv", WvT, HID)
                    vT_sb = load(vT_p, "vTin", vT, LK)
                    # Wo reuses the Wv slots: its DMAs start as V-proj
                    # consumes each Wv tile for the last time.
                    wo_sb = load(wv_p, "wv", WoT, HID)
                    emit_qproj(wq_sb, qT_sb)
                pt_p = pool(name="pt", bufs=17)
                nrm_p = pool(name="nrm", bufs=1)

                khT[0] = emit_kproj(0, 0, 0, wk_sb, kT_sb, khT_p, "khT")
                khT[1] = emit_kproj(1, 1, 1, wk_sb, kT_sb, khT_p, "khT")

                # remaining khT tiles are produced by short filler units
                # (8 accumulating matmuls + immediate evac) woven into the
                # attention j-loop so the PSUM slots keep rotating and
                # ScalarE never starves behind a long K-projection block
                import collections
                fillers = collections.deque()
                for ot in range(2, OT):
                    t = khT_p.tile([P, LK], bf16, tag="khT", name=f"khT{ot}")
                    khT[ot] = t
                    for c0 in range(0, LK, 512):
                        def kunit(ot=ot, c0=c0, t=t):
                            w = min(512, LK - c0)
                            ps = psum.tile([P, 1024], f32, tag="mm", bufs=2,
                                           name=f"kps{ot}_{c0}")
                            for i in range(KT):
                                nc.tensor.matmul(
                                    ps[:, 0:w],
                                    wk_sb[i][:, ot * P : (ot + 1) * P],
                                    kT_sb[i][:, c0 : c0 + w],
                                    start=(i == 0),
                                    stop=(i == KT - 1),
                                )
                            nc.vector.tensor_scalar(
                                t[:, c0 : c0 + w], ps[:, 0:w],
                                bk_sb[:, ot : ot + 1], None, op0=ADD,
                            )
                        fillers.append(kunit)

                def qkexp_one(hp, j):
                    sp = psum.tile([P, 1024], f32, tag="mm", bufs=2,
                                   name=f"sp{hp}_{j}")
                    nc.tensor.matmul(
                        sp[:, 0:512],
                        khT[hp][0:DH, j * P : (j + 1) * P],
                        qhT[hp][0:DH, :],
                        start=True, stop=True,
                    )
                    nc.tensor.matmul(
                        sp[:, 512:1024],
                        khT[hp][DH:P, j * P : (j + 1) * P],
                        qhT[hp][DH:P, :],
                        start=True, stop=True,
                    )
                    pt = pt_p.tile([P, 1024], bf16, tag="pt",
                                   name=f"pt{hp}_{j}")
                    nc.scalar.activation(
                        pt[:], sp[:], EXP,
                        bias=maskb_sb[:, j : j + 1], scale=SCALE,
                    )
                    pts[(hp, j)] = pt

                # PV accumulation for pair hp, woven per-j with pair hp+1
                # QK/exp, the deferred tail of pair hp-1, and K-projection
                # filler units
                def emit_pair_step(hp, prev_tail):
                    nxt = hp + 1
                    pv0 = psum.tile([VW, LQ], f32, tag="pv", bufs=4,
                                    name=f"pv0h{hp}")
                    pv1 = psum.tile([VW, LQ], f32, tag="pv", bufs=4,
                                    name=f"pv1h{hp}")
                    tail_j = min(2, LTK - 1)
                    for j in range(LTK):
                        if nxt < NPAIR:
                            qkexp_one(nxt, j)
                        pt = pts.pop((hp, j))
                        nc.tensor.matmul(
                            pv0[:, :],
                            vhx[j][:, (2 * hp) * VW : (2 * hp + 1) * VW],
                            pt[:, 0:512],
                            start=(j == 0),
                            stop=(j == LTK - 1),
                        )
                        nc.tensor.matmul(
                            pv1[:, :],
                            vhx[j][:, (2 * hp + 1) * VW : (2 * hp + 2) * VW],
                            pt[:, 512:1024],
                            start=(j == 0),
                            stop=(j == LTK - 1),
                        )
                        if j == tail_j and prev_tail is not None:
                            prev_tail()
                        # 2 filler units per step (not 3): spreads the
                        # K-projection work into steps 4-5 where ACT binds
                        # and PE otherwise idles ~0.9us/step
                        if fillers and j % 4 == 1:
                            fillers.popleft()()
                    if hp == NPAIR - 2:
                        while fillers:
                            fillers.popleft()()
                    return make_tail(hp, pv0, pv1)

                emit_qkexp(0, vstep=lambda j: emit_vstep(j, wv_sb, vT_sb))
                tail = None
                for hp in range(NPAIR):
                    tail = emit_pair_step(hp, tail)
                tail()
                emit_oproj(wo_sb)
            else:
                # upfront fallback (any LTK): unsharded projections with
                # stack-scoped pools so SBUF fits at full key count
                with (
                    tc.tile_pool(name="wq", bufs=KT) as wq_p,
                    tc.tile_pool(name="qTin", bufs=KT) as qT_p,
                ):
                    emit_qproj(load(wq_p, "wq", WqT, HID),
                               load(qT_p, "qTin", qT, LQ))
                with (
                    tc.tile_pool(name="wk", bufs=KT) as wk_p,
                    tc.tile_pool(name="kin", bufs=KT) as kT_p,
                ):
                    wk_sb = load(wk_p, "wk", WkT, HID)
                    kT_sb = load(kT_p, "kTin", kT, LK)
                    for ot in range(OT):
                        khT[ot] = emit_kproj(ot, ot, ot, wk_sb, kT_sb,
                                             khT_p, "khT")
                with (
                    tc.tile_pool(name="wv", bufs=KT) as wv_p,
                    tc.tile_pool(name="vin", bufs=KT) as vT_p,
                ):
                    wv_sb = load(wv_p, "wv", WvT, HID)
                    vT_sb = load(vT_p, "vTin", vT, LK)
                    for j in range(LTK):
                        emit_vstep(j, wv_sb, vT_sb)
                pt_p = pool(name="pt", bufs=17)
                nrm_p = pool(name="nrm", bufs=1)
                emit_qkexp(0)
                tail = None
                for hp in range(NPAIR):
                    if hp + 1 < NPAIR:
                        emit_qkexp(hp + 1)
                    tail = emit_pv(hp, prev_tail=tail)
                tail()
                with tc.tile_pool(name="wo", bufs=KT) as wo_p:
                    emit_oproj(load(wo_p, "wo", WoT, HID))

    nc.compile()
    return nc


def make_in_maps(q, k, v, mask, Wq, bq, Wk, bk, Wv, bv, Wo, bo):
    """Shard + lay out the full inputs for the 8 cores (host-side numpy).
    Compacts K/V to the unmasked keys per batch (masked keys contribute
    exactly 0 to the reference softmax). Returns (in_maps, LTK)."""
    q = np.asarray(q, np.float32)
    k = np.asarray(k, np.float32)
    v = np.asarray(v, np.float32)
    mask = np.asarray(mask)

    keep = [np.flatnonzero(mask[b, 0, 0, :] != 0) for b in range(B)]
    nmax = max(1, max(len(kp) for kp in keep))
    LK = P * ((nmax + P - 1) // P)
    LTK = LK // P

    def t_bf16(a):  # [R, C] -> contiguous [C, R] bf16
        return np.ascontiguousarray(np.asarray(a, np.float32).T).astype(BF16)

    WqT_h, WkT_h, WvT_h, WoT_h = (t_bf16(w) for w in (Wq, Wk, Wv, Wo))

    def b_tiles(b):  # [HID] -> [128, 8] f32 (per-o-tile partition vectors)
        return np.ascontiguousarray(
            np.asarray(b, np.float32).reshape(OT, P).T
        )

    bq_h, bk_h, bo_h = b_tiles(bq), b_tiles(bk), b_tiles(bo)
    bv_h = np.asarray(bv, np.float32)[None, :].astype(BF16)

    per_batch = {}
    for b in range(B):
        nb = len(keep[b])
        kc = np.zeros((LK, HID), np.float32)
        vc = np.zeros((LK, HID), np.float32)
        kc[:nb] = k[b][keep[b]]
        vc[:nb] = v[b][keep[b]]
        mb = np.zeros(LK, np.float32)
        mb[nb:] = -10000.0
        maskb_h = np.ascontiguousarray(mb.reshape(LTK, P).T)
        per_batch[b] = (t_bf16(kc), t_bf16(vc), maskb_h)

    in_maps = []
    for c in range(N_CORES):
        b, s = divmod(c, GSZ)
        r0 = s * LQ
        kT_h, vT_h, maskb_h = per_batch[b]
        m = {
            "qT": t_bf16(q[b, r0 : r0 + LQ, :]),
            "kT": kT_h,
            "vT": vT_h,
            "WqT": WqT_h,
            "WvT": WvT_h,
            "WoT": WoT_h,
            "bq": bq_h,
            "bo": bo_h,
            "bv_row": bv_h,
            "maskb": maskb_h,
        }
        m["WkT"] = WkT_h
        m["bk"] = bk_h
        in_maps.append(m)
    return in_maps, LTK


def assemble_output(results):
    """Gather per-core out.T [HID, LQ] slices into the full [B, L, HID]."""
    full = np.empty((B, L, HID), np.float32)
    for c in range(N_CORES):
        b, ch = divmod(c, GSZ)
        r0 = ch * LQ
        full[b, r0 : r0 + LQ, :] = results[c]["out"].T
    return full


_NC_CACHE = {}


def _run(trace=False, **inputs):
    _ensure_profile_hook()
    from concourse.bass_utils import run_bass_kernel_spmd
    from concourse import bass_utils

    bass_utils.upload_artifacts = lambda tmpdir: tmpdir  # zero-egress container
    in_maps, LTK = make_in_maps(**inputs)
    if LTK not in _NC_CACHE:
        _NC_CACHE[LTK] = build_bass(LTK)
    res = run_bass_kernel_spmd(
        _NC_CACHE[LTK], in_maps, core_ids=list(range(N_CORES)), trace=trace
    )
    return assemble_output(res.results), res


def kernel(**inputs):
    out, _ = _run(trace=False, **inputs)
    return out


# revision 29
# speedup vs baseline: 1.2245x; 1.2245x over previous
"""Distributed Trainium2 kernel for a 16-head attention layer.

Problem: B=2, L=2048, HID=1024, H=16 (torch-Linear projections, masked softmax).
Sharding: 8 cores = batch (2) x query-chunk (4). Each core computes the Q
projection for its 512-query chunk, attention for all 16 heads over its
queries, the output projection for its chunk, and the full K/V projections
for its batch (duplicated within the 4-core group -- collectives on this
harness pay the full cross-core launch skew, measured ~30us per call, so
the kernel stays embarrassingly parallel).

Optimizations over the v0 baseline:
- Masked-key compaction: mask is [B,1,1,L] and masked keys score -10000 ->
  exp underflows to exactly 0 in f32 (same as the reference's softmax), so
  they contribute nothing to numerator or denominator. The host compacts
  K/V/mask to the kept keys (padded to a multiple of 128; pad slots get the
  -10000 bias). With the ~Bernoulli(0.5) mask this halves K/V projection,
  QK, PV and ScalarE-exp work. The Bass program is compiled per
  key-tile-count (LTK) and cached; LTK > 10 falls back to an unsharded
  upfront schedule that fits SBUF at any key count.
- Pipelined attention: QK/exp for pair hp+1 is emitted before the PV
  accumulation of pair hp (lag-1 ladder), so the in-order engine queues
  pipeline the QK -> exp -> PV chain instead of serializing it; the
  normalize tail of each pair is deferred into the next pair's PV phase so
  the PE never stalls on the DVE reciprocal chain.
- Normalize tail: denominators (PSUM row 64 of the PV accumulators, from
  the ones-column trick) are staged to SBUF, inverted with the custom-DVE
  reciprocal_approx_fast (~5x faster than the iterative divide), and
  broadcast across the 64 head dims with two concurrent column-tiled fp16
  rank-1 matmuls (the v0 fp32 ones-matmuls ran at quarter rate).

Layout strategy: all matmul contractions need the contracted dim on SBUF
partitions, so activations and weights are fed pre-transposed (host-side
numpy transpose + bf16 cast during sharding):
  qT [HID, LQ]  kT/vT [HID, LK]   W*T [HID, HID] (= W.T)
Scores are computed transposed (S.T[lk, lq]) so the mask bias is a
per-partition bias fused into the ScalarE exp, and PV consumes P.T directly.
A ones-column appended to each V tile makes the PV matmul emit the softmax
denominators as PSUM row 64.
"""

import sys
import types

import numpy as np
import ml_dtypes

# ---- problem constants (hardcoded; kernel.py must be self-contained) ----
B, L, HID, H = 2, 2048, 1024, 16
DH = HID // H          # 64
N_CORES = 8
GSZ = N_CORES // B     # 4 cores per batch group
LQ = (B * L) // N_CORES  # 512 queries per core
P = 128
KT = HID // P          # 8 contraction tiles
OT = HID // P          # 8 output tiles
NPAIR = H // 2         # 8 head pairs
SCALE = DH ** -0.5
BF16 = ml_dtypes.bfloat16
VW = DH + 1            # 65: per-head V columns + ones column for denominator


def _ensure_profile_hook():
    """Install the NTFF profiling hook trn_boot couldn't (antenv.axon_hooks
    is missing from the image); harmless if profiling is never requested."""
    if "antenv.axon_hooks" in sys.modules:
        return
    try:
        from trn_agent_boot.trn_boot import _ntff_profile_via_ctypes

        hook = _ntff_profile_via_ctypes("/opt/axon/libaxon_pjrt.so")
    except Exception:
        hook = None
    mod = types.ModuleType("antenv.axon_hooks")
    mod.get_axon_ntff_profile_hook = lambda: hook
    mod.set_axon_ntff_profile_hook = lambda h: None
    sys.modules["antenv.axon_hooks"] = mod


def build_bass(LTK):
    """Build + compile the per-core Bass program for LTK key tiles
    (LK = 128*LTK compacted keys; same graph on all 8 cores)."""
    import concourse.mybir as mybir
    import concourse.tile as tile
    from concourse import bacc
    import contextlib

    LK = LTK * P
    interleaved = LTK <= 10
    f32 = mybir.dt.float32
    bf16 = mybir.dt.bfloat16
    fp16 = mybir.dt.float16
    ADD = mybir.AluOpType.add
    MULT = mybir.AluOpType.mult
    BYPASS = mybir.AluOpType.bypass
    EXP = mybir.ActivationFunctionType.Exp
    RG = [list(range(g * GSZ, (g + 1) * GSZ)) for g in range(B)]

    nc = bacc.Bacc("TRN2", target_bir_lowering=False, debug=False, num_devices=N_CORES)

    qT = nc.declare_dram_parameter("qT", [HID, LQ], bf16, isOutput=False)
    kT = nc.declare_dram_parameter("kT", [HID, LK], bf16, isOutput=False)
    vT = nc.declare_dram_parameter("vT", [HID, LK], bf16, isOutput=False)
    WqT = nc.declare_dram_parameter("WqT", [HID, HID], bf16, isOutput=False)
    WvT = nc.declare_dram_parameter("WvT", [HID, HID], bf16, isOutput=False)
    WoT = nc.declare_dram_parameter("WoT", [HID, HID], bf16, isOutput=False)
    bq = nc.declare_dram_parameter("bq", [P, OT], f32, isOutput=False)
    bo = nc.declare_dram_parameter("bo", [P, OT], f32, isOutput=False)
    bv_row = nc.declare_dram_parameter("bv_row", [1, HID], bf16, isOutput=False)
    maskb = nc.declare_dram_parameter("maskb", [P, LTK], f32, isOutput=False)
    out = nc.declare_dram_parameter("out", [HID, LQ], f32, isOutput=True)
    WkT = nc.declare_dram_parameter("WkT", [HID, HID], bf16, isOutput=False)
    bk = nc.declare_dram_parameter("bk", [P, OT], f32, isOutput=False)

    with tile.TileContext(nc) as tc:
        with contextlib.ExitStack() as _stk:
            pool = lambda *a, **k: _stk.enter_context(tc.tile_pool(*a, **k))
            consts = pool(name="consts", bufs=1)
            khT_p = pool(name="khT", bufs=OT)
            vhx_p = pool(name="vhx", bufs=LTK)
            qhT_p = pool(name="qhT", bufs=OT)
            attnT_p = pool(name="attnT", bufs=NPAIR)
            osb_p = pool(name="osb", bufs=2)
            psum = pool(name="psum", bufs=1, space="PSUM")

            # ---- constants ----
            maskb_sb = consts.tile([P, LTK], f32)
            nc.sync.dma_start(maskb_sb[:], maskb[:])
            bq_sb = consts.tile([P, OT], f32, tag="bq")
            nc.sync.dma_start(bq_sb[:], bq[:])
            bo_sb = consts.tile([P, OT], f32, tag="bo")
            nc.sync.dma_start(bo_sb[:], bo[:])
            bvr_sb = consts.tile([1, HID], bf16, tag="bvr")
            nc.sync.dma_start(bvr_sb[:], bv_row[:])
            ones_bf = consts.tile([1, P], bf16, tag="ones_bf")
            nc.vector.memset(ones_bf[:], 1.0)
            ones16 = consts.tile([1, DH], fp16, tag="ones16")
            nc.vector.memset(ones16[:], 1.0)
            bk_sb = consts.tile([P, OT], f32, tag="bk")
            nc.sync.dma_start(bk_sb[:], bk[:])

            # bias-v broadcast tile [128, HID] (bias along free dim needs a
            # full tile; built once via a rank-1 ones @ bv_row matmul)
            bvb_ps = psum.tile([P, 1024], f32, tag="mm", bufs=2)
            for h2 in range(2):
                nc.tensor.matmul(
                    bvb_ps[:, h2 * 512 : (h2 + 1) * 512],
                    ones_bf[:, :],
                    bvr_sb[:, h2 * 512 : (h2 + 1) * 512],
                    start=True,
                    stop=True,
                )
            bvb = consts.tile([P, HID], f32, tag="bvb")
            nc.vector.tensor_copy(bvb[:], bvb_ps[:])

            khT = [None] * OT
            vhx = [None] * LTK
            qhT = [None] * OT
            attnT = [None] * NPAIR

            def load(pool_, tag, dramp, width):
                tiles = []
                for i in range(KT):
                    w = pool_.tile([P, width], bf16, tag=tag, name=f"{tag}{i}")
                    nc.sync.dma_start(w[:], dramp[i * P : (i + 1) * P, :])
                    tiles.append(w)
                return tiles

            def emit_qproj(wq_sb, qT_sb):
                for ot in range(OT):
                    ps = psum.tile([P, 1024], f32, tag="mm", bufs=2,
                                   name=f"qps{ot}")
                    for i in range(KT):
                        nc.tensor.matmul(
                            ps[:, 0:LQ],
                            wq_sb[i][:, ot * P : (ot + 1) * P],
                            qT_sb[i][:, :],
                            start=(i == 0),
                            stop=(i == KT - 1),
                        )
                    t = qhT_p.tile([P, LQ], bf16, tag="qhT")
                    nc.vector.tensor_scalar(
                        t[:], ps[:, 0:LQ], bq_sb[:, ot : ot + 1], None, op0=ADD
                    )
                    qhT[ot] = t

            def emit_kproj(ot, wcol, bcol, wk_sb, kT_sb, dst_p, dst_tag):
                """One khT-style output tile [128, LK] bf16 from weight
                column block `wcol` of wk_sb, bias column `bcol`."""
                t = dst_p.tile([P, LK], bf16, tag=dst_tag, name=f"{dst_tag}{ot}")
                for c0 in range(0, LK, 1024):
                    w = min(1024, LK - c0)
                    ps = psum.tile([P, 1024], f32, tag="mm", bufs=2,
                                   name=f"kps{dst_tag}{ot}_{c0}")
                    for h0 in range(0, w, 512):
                        hw = min(512, w - h0)
                        for i in range(KT):
                            nc.tensor.matmul(
                                ps[:, h0 : h0 + hw],
                                wk_sb[i][:, wcol * P : (wcol + 1) * P],
                                kT_sb[i][:, c0 + h0 : c0 + h0 + hw],
                                start=(i == 0),
                                stop=(i == KT - 1),
                            )
                    nc.vector.tensor_scalar(
                        t[:, c0 : c0 + w],
                        ps[:, 0:w],
                        bk_sb[:, bcol : bcol + 1],
                        None,
                        op0=ADD,
                    )
                return t

            def emit_vstep(j, wv_sb, vT_sb):
                """vhx[j] [128, 16*65] bf16 (keys on partitions, per-head
                64 value cols + ones col)."""
                ps = psum.tile([P, 1024], f32, tag="mm", bufs=2, name=f"vps{j}")
                for half in range(2):
                    for i in range(KT):
                        nc.tensor.matmul(
                            ps[:, half * 512 : (half + 1) * 512],
                            vT_sb[i][:, j * P : (j + 1) * P],
                            wv_sb[i][:, half * 512 : (half + 1) * 512],
                            start=(i == 0),
                            stop=(i == KT - 1),
                        )
                t = vhx_p.tile([P, H * VW], bf16, tag="vhx", name=f"vhx{j}")
                ps3 = ps[:].rearrange("p (h d) -> p h d", d=DH)
                out3 = t[:].rearrange("p (h w) -> p h w", w=VW)[:, :, 0:DH]
                bvb3 = bvb[:].rearrange("p (h d) -> p h d", d=DH)
                nc.vector.tensor_tensor(out3, ps3, bvb3, op=ADD)
                ones3 = t[:].rearrange("p (h w) -> p h w", w=VW)[:, :, DH:VW]
                nc.vector.memset(ones3, 1.0)
                vhx[j] = t

            # ---- attention: QK/exp runs a one-pair ladder ahead of PV ----
            pts = {}

            def emit_qkexp(hp, vstep=None):
                for j in range(LTK):
                    if vstep is not None:
                        vstep(j)
                    sp = psum.tile([P, 1024], f32, tag="mm", bufs=2,
                                   name=f"sp{hp}_{j}")
                    nc.tensor.matmul(
                        sp[:, 0:512],
                        khT[hp][0:DH, j * P : (j + 1) * P],
                        qhT[hp][0:DH, :],
                        start=True,
                        stop=True,
                    )
                    nc.tensor.matmul(
                        sp[:, 512:1024],
                        khT[hp][DH:P, j * P : (j + 1) * P],
                        qhT[hp][DH:P, :],
                        start=True,
                        stop=True,
                    )
                    pt = pt_p.tile([P, 1024], bf16, tag="pt", name=f"pt{hp}_{j}")
                    nc.scalar.activation(
                        pt[:], sp[:], EXP,
                        bias=maskb_sb[:, j : j + 1], scale=SCALE,
                    )
                    pts[(hp, j)] = pt

            def make_tail(hp, pv0, pv1):
                """Deferred normalize tail for pair hp (emitted mid-way
                through the next pair's PV phase so the in-order PE queue
                never stalls on the DVE reciprocal chain)."""
                def tail():
                    dsb = nrm_p.tile([1, 2 * LQ], f32, tag="dsb",
                                     name=f"dsbh{hp}")
                    nc.vector.tensor_copy(dsb[:, 0:LQ], pv0[DH : DH + 1, :])
                    nc.vector.tensor_copy(dsb[:, LQ : 2 * LQ],
                                          pv1[DH : DH + 1, :])
                    rc32 = nrm_p.tile([1, 2 * LQ], f32, tag="rc32",
                                      name=f"rc32h{hp}")
                    nc.vector.reciprocal_approx_fast(rc32[:], dsb[:])
                    rc16 = nrm_p.tile([1, 2 * LQ], fp16, tag="rc16",
                                      name=f"rc16h{hp}")
                    nc.vector.tensor_copy(rc16[:], rc32[:])
                    rb = psum.tile([P, 1024], f32, tag="mm", bufs=2,
                                   name=f"rb{hp}")
                    nc.tensor.matmul(
                        rb[0:DH, 0:LQ], ones16[:, :], rc16[:, 0:LQ],
                        start=True, stop=True, tile_position=(0, 0),
                    )
                    nc.tensor.matmul(
                        rb[DH:P, 0:LQ], ones16[:, :], rc16[:, LQ : 2 * LQ],
                        start=True, stop=True, tile_position=(0, DH),
                    )
                    # DVE has a single PSUM port: stage the broadcast in SBUF
                    # so the normalize mult reads one PSUM operand
                    rbs = nrm_p.tile([P, LQ], f32, tag="rbs", name=f"rbs{hp}")
                    nc.vector.tensor_copy(rbs[:], rb[:, 0:LQ])
                    at = attnT_p.tile([P, LQ], bf16, tag="attnT",
                                      name=f"at{hp}")
                    nc.vector.tensor_tensor(
                        at[0:DH, :], pv0[0:DH, :], rbs[0:DH, :], op=MULT
                    )
                    nc.vector.tensor_tensor(
                        at[DH:P, :], pv1[0:DH, :], rbs[DH:P, :], op=MULT
                    )
                    attnT[hp] = at
                return tail

            def emit_pv(hp, prev_tail=None):
                pv0 = psum.tile([VW, LQ], f32, tag="pv", bufs=4,
                                name=f"pv0h{hp}")
                pv1 = psum.tile([VW, LQ], f32, tag="pv", bufs=4,
                                name=f"pv1h{hp}")
                tail_j = min(2, LTK - 1)
                for j in range(LTK):
                    pt = pts.pop((hp, j))
                    nc.tensor.matmul(
                        pv0[:, :],
                        vhx[j][:, (2 * hp) * VW : (2 * hp + 1) * VW],
                        pt[:, 0:512],
                        start=(j == 0),
                        stop=(j == LTK - 1),
                    )
                    nc.tensor.matmul(
                        pv1[:, :],
                        vhx[j][:, (2 * hp + 1) * VW : (2 * hp + 2) * VW],
                        pt[:, 512:1024],
                        start=(j == 0),
                        stop=(j == LTK - 1),
                    )
                    if j == tail_j and prev_tail is not None:
                        prev_tail()
                return make_tail(hp, pv0, pv1)

            def emit_oproj(wo_sb):
                for ot in range(OT):
                    ps = psum.tile([P, 1024], f32, tag="mm", bufs=2,
                                   name=f"ops{ot}")
                    for i in range(KT):
                        nc.tensor.matmul(
                            ps[:, 0:LQ],
                            wo_sb[i][:, ot * P : (ot + 1) * P],
                            attnT[i][:, :],
                            start=(i == 0),
                            stop=(i == KT - 1),
                        )
                    o = osb_p.tile([P, LQ], f32, tag="osb")
                    nc.vector.tensor_scalar(
                        o[:], ps[:, 0:LQ], bo_sb[:, ot : ot + 1], None, op0=ADD
                    )
                    nc.sync.dma_start(out[ot * P : (ot + 1) * P, :], o[:])

            if interleaved:
                # Q projection first (its 3MB of inputs gate the first PE
                # work); scoped pools so the stack allocator reuses their
                # SBUF region for the pt/nrm pools opened after
                wk_p = pool(name="wk", bufs=KT)
                kT_p = pool(name="kin", bufs=KT)
                wv_p = pool(name="wv", bufs=KT)
                vT_p = pool(name="vin", bufs=KT)
                with (
                    tc.tile_pool(name="wq", bufs=KT) as wq_p,
                    tc.tile_pool(name="qTin", bufs=KT) as qT_p,
                ):
                    wq_sb = load(wq_p, "wq", WqT, HID)
                    qT_sb = load(qT_p, "qTin", qT, LQ)
                    wk_sb = load(wk_p, "wk", WkT, HID)
                    kT_sb = load(kT_p, "kTin", kT, LK)
                    wv_sb = load(wv_p, "wv", WvT, HID)
                    vT_sb = load(vT_p, "vTin", vT, LK)
                    # Wo reuses the Wv slots: its DMAs start as V-proj
                    # consumes each Wv tile for the last time.
                    wo_sb = load(wv_p, "wv", WoT, HID)
                    emit_qproj(wq_sb, qT_sb)
                pt_p = pool(name="pt", bufs=17)
                nrm_p = pool(name="nrm", bufs=1)

                khT[0] = emit_kproj(0, 0, 0, wk_sb, kT_sb, khT_p, "khT")
                khT[1] = emit_kproj(1, 1, 1, wk_sb, kT_sb, khT_p, "khT")

                # remaining khT tiles are produced by short filler units
                # (8 accumulating matmuls + immediate evac) woven into the
                # attention j-loop so the PSUM slots keep rotating and
                # ScalarE never starves behind a long K-projection block
                import collections
                fillers = collections.deque()
                for ot in range(2, OT):
                    t = khT_p.tile([P, LK], bf16, tag="khT", name=f"khT{ot}")
                    khT[ot] = t
                    for c0 in range(0, LK, 512):
                        def kunit(ot=ot, c0=c0, t=t):
                            w = min(512, LK - c0)
                            ps = psum.tile([P, 1024], f32, tag="mm", bufs=2,
                                           name=f"kps{ot}_{c0}")
                            for i in range(KT):
                                nc.tensor.matmul(
                                    ps[:, 0:w],
                                    wk_sb[i][:, ot * P : (ot + 1) * P],
                                    kT_sb[i][:, c0 : c0 + w],
                                    start=(i == 0),
                                    stop=(i == KT - 1),
                                )
                            nc.vector.tensor_scalar(
                                t[:, c0 : c0 + w], ps[:, 0:w],
                                bk_sb[:, ot : ot + 1], None, op0=ADD,
                            )
                        fillers.append(kunit)

                def qkexp_one(hp, j):
                    sp = psum.tile([P, 1024], f32, tag="mm", bufs=2,
                                   name=f"sp{hp}_{j}")
                    nc.tensor.matmul(
                        sp[:, 0:512],
                        khT[hp][0:DH, j * P : (j + 1) * P],
                        qhT[hp][0:DH, :],
                        start=True, stop=True,
                    )
                    nc.tensor.matmul(
                        sp[:, 512:1024],
                        khT[hp][DH:P, j * P : (j + 1) * P],
                        qhT[hp][DH:P, :],
                        start=True, stop=True,
                    )
                    pt = pt_p.tile([P, 1024], bf16, tag="pt",
                                   name=f"pt{hp}_{j}")
                    nc.scalar.activation(
                        pt[:], sp[:], EXP,
                        bias=maskb_sb[:, j : j + 1], scale=SCALE,
                    )
                    pts[(hp, j)] = pt

                # PV accumulation for pair hp, woven per-j with pair hp+1
                # QK/exp, the deferred tail of pair hp-1, and K-projection
                # filler units
                def emit_pair_step(hp, prev_tail):
                    nxt = hp + 1
                    pv0 = psum.tile([VW, LQ], f32, tag="pv", bufs=4,
                                    name=f"pv0h{hp}")
                    pv1 = psum.tile([VW, LQ], f32, tag="pv", bufs=4,
                                    name=f"pv1h{hp}")
                    tail_j = min(2, LTK - 1)
                    for j in range(LTK):
                        if nxt < NPAIR:
                            qkexp_one(nxt, j)
                        pt = pts.pop((hp, j))
                        nc.tensor.matmul(
                            pv0[:, :],
                            vhx[j][:, (2 * hp) * VW : (2 * hp + 1) * VW],
                            pt[:, 0:512],
                            start=(j == 0),
                            stop=(j == LTK - 1),
                        )
                        nc.tensor.matmul(
                            pv1[:, :],
                            vhx[j][:, (2 * hp + 1) * VW : (2 * hp + 2) * VW],
                            pt[:, 512:1024],
                            start=(j == 0),
                            stop=(j == LTK - 1),
                        )
                        if j == tail_j and prev_tail is not None:
                            prev_tail()
                        if fillers and j % 3 == 1:
                            fillers.popleft()()
                    if hp == NPAIR - 2:
                        while fillers:
                            fillers.popleft()()
                    return make_tail(hp, pv0, pv1)

                emit_qkexp(0, vstep=lambda j: emit_vstep(j, wv_sb, vT_sb))
                tail = None
                for hp in range(NPAIR):
                    tail = emit_pair_step(hp, tail)
                tail()
                emit_oproj(wo_sb)
            else:
                # upfront fallback (any LTK): unsharded projections with
                # stack-scoped pools so SBUF fits at full key count
                with (
                    tc.tile_pool(name="wq", bufs=KT) as wq_p,
                    tc.tile_pool(name="qTin", bufs=KT) as qT_p,
                ):
                    emit_qproj(load(wq_p, "wq", WqT, HID),
                               load(qT_p, "qTin", qT, LQ))
                with (
                    tc.tile_pool(name="wk", bufs=KT) as wk_p,
                    tc.tile_pool(name="kin", bufs=KT) as kT_p,
                ):
                    wk_sb = load(wk_p, "wk", WkT, HID)
                    kT_sb = load(kT_p, "kTin", kT, LK)
                    for ot in range(OT):
                        khT[ot] = emit_kproj(ot, ot, ot, wk_sb, kT_sb,
                                             khT_p, "khT")
                with (
                    tc.tile_pool(name="wv", bufs=KT) as wv_p,
                    tc.tile_pool(name="vin", bufs=KT) as vT_p,
                ):
                    wv_sb = load(wv_p, "wv", WvT, HID)
                    vT_sb = load(vT_p, "vTin", vT, LK)
                    for j in range(LTK):
                        emit_vstep(j, wv_sb, vT_sb)
                pt_p = pool(name="pt", bufs=17)
                nrm_p = pool(name="nrm", bufs=1)
                emit_qkexp(0)
                tail = None
                for hp in range(NPAIR):
                    if hp + 1 < NPAIR:
                        emit_qkexp(hp + 1)
                    tail = emit_pv(hp, prev_tail=tail)
                tail()
                with tc.tile_pool(name="wo", bufs=KT) as wo_p:
                    emit_oproj(load(wo_p, "wo", WoT, HID))

    nc.compile()
    return nc


def make_in_maps(q, k, v, mask, Wq, bq, Wk, bk, Wv, bv, Wo, bo):
    """Shard + lay out the full inputs for the 8 cores (host-side numpy).
    Compacts K/V to the unmasked keys per batch (masked keys contribute
    exactly 0 to the reference softmax). Returns (in_maps, LTK)."""
    q = np.asarray(q, np.float32)
    k = np.asarray(k, np.float32)
    v = np.asarray(v, np.float32)
    mask = np.asarray(mask)

    keep = [np.flatnonzero(mask[b, 0, 0, :] != 0) for b in range(B)]
    nmax = max(1, max(len(kp) for kp in keep))
    LK = P * ((nmax + P - 1) // P)
    LTK = LK // P

    def t_bf16(a):  # [R, C] -> contiguous [C, R] bf16
        return np.ascontiguousarray(np.asarray(a, np.float32).T).astype(BF16)

    WqT_h, WkT_h, WvT_h, WoT_h = (t_bf16(w) for w in (Wq, Wk, Wv, Wo))

    def b_tiles(b):  # [HID] -> [128, 8] f32 (per-o-tile partition vectors)
        return np.ascontiguousarray(
            np.asarray(b, np.float32).reshape(OT, P).T
        )

    bq_h, bk_h, bo_h = b_tiles(bq), b_tiles(bk), b_tiles(bo)
    bv_h = np.asarray(bv, np.float32)[None, :].astype(BF16)

    per_batch = {}
    for b in range(B):
        nb = len(keep[b])
        kc = np.zeros((LK, HID), np.float32)
        vc = np.zeros((LK, HID), np.float32)
        kc[:nb] = k[b][keep[b]]
        vc[:nb] = v[b][keep[b]]
        mb = np.zeros(LK, np.float32)
        mb[nb:] = -10000.0
        maskb_h = np.ascontiguousarray(mb.reshape(LTK, P).T)
        per_batch[b] = (t_bf16(kc), t_bf16(vc), maskb_h)

    in_maps = []
    for c in range(N_CORES):
        b, s = divmod(c, GSZ)
        r0 = s * LQ
        kT_h, vT_h, maskb_h = per_batch[b]
        m = {
            "qT": t_bf16(q[b, r0 : r0 + LQ, :]),
            "kT": kT_h,
            "vT": vT_h,
            "WqT": WqT_h,
            "WvT": WvT_h,
            "WoT": WoT_h,
            "bq": bq_h,
            "bo": bo_h,
            "bv_row": bv_h,
            "maskb": maskb_h,
        }
        m["WkT"] = WkT_h
        m["bk"] = bk_h
        in_maps.append(m)
    return in_maps, LTK


def assemble_output(results):
    """Gather per-core out.T [HID, LQ] slices into the full [B, L, HID]."""
    full = np.empty((B, L, HID), np.float32)
    for c in range(N_CORES):
        b, ch = divmod(c, GSZ)
        r0 = ch * LQ
        full[b, r0 : r0 + LQ, :] = results[c]["out"].T
    return full


_NC_CACHE = {}


def _run(trace=False, **inputs):
    _ensure_profile_hook()
    from concourse.bass_utils import run_bass_kernel_spmd
    from concourse import bass_utils

    bass_utils.upload_artifacts = lambda tmpdir: tmpdir  # zero-egress container
    in_maps, LTK = make_in_maps(**inputs)
    if LTK not in _NC_CACHE:
        _NC_CACHE[LTK] = build_bass(LTK)
    res = run_bass_kernel_spmd(
        _NC_CACHE[LTK], in_maps, core_ids=list(range(N_CORES)), trace=trace
    )
    return assemble_output(res.results), res


def kernel(**inputs):
    out, _ = _run(trace=False, **inputs)
    return out
